# revision 9
# baseline (speedup 1.0000x reference)
"""Trainium2 Bass kernel for the CNN-TRX few-shot attention head.

Sharding: data-parallel over the 200 queries (25 per NeuronCore); support set
and weights replicated per core. All matmuls in bf16 with fp32 PSUM.

  1. Frame projection in transposed layout: f_T[d, frame] for all 6 weight
     blocks (k_w/v_w x 3 tuple positions); biases folded via an augmented
     ones-row of X only when biases are nonzero (16 vs 17 K-chunks).
  2. Tuple gather (C(8,3)=56 frame triples) as 2-stage DVE column adds, one
     merged n_items=25 call per path/side (amortizes DVE dispatch).
  3. Support rows class-sorted UNPADDED (1400 -> 1408 cols, 11 chunks).
     Per-class column sums use per-(class,chunk) indicator columns (input
     `ind`) as M=1 matmul lhsT; prototypes use raw s_v chunks plus
     mask-multiplied copies of the class-boundary chunks.
  4. LayerNorm of K projections column-wise: stats via two ones-matmul
     streams packed into concurrent PE column groups, Rsqrt on ACT, gpsimd
     partition-broadcast, two DVE passes.
  5. scoresT = s_k^T q_k per 128-row chunk; exp via ACT (no max-subtract:
     LN'd scores are bounded, exp stays finite in fp32).
  6. Query groups of 9/9/7 queries (504/504/392 score columns) pipelined
     through double-buffered PSUM/SBUF pools; distance terms ||q_v||^2,
     <q_v,P>, ||P||^2, sum(exp) via column-sum matmuls packed 4-per-PSUM
     tile at partitions {0,32,64,96}; final combine on 5 class lanes;
     logits = -sum_a dist / 56.
"""

import math
from itertools import combinations

import ml_dtypes
import numpy as np

SEQ = 8
IN_DIM = 2048
OUT_DIM = 1152
TSS = 3
WAY = 5
N_SUPPORT = 25
N_QUERIES = 200
PE_SCALE = 0.1
LN_EPS = 1e-5
T = 56
N_CORES = 8
NQL = N_QUERIES // N_CORES      # queries per core
NDCH = OUT_DIM // 128           # 9
NMB = 6 * OUT_DIM // 128        # 54 projection column blocks
NX = SEQ * 2 * N_SUPPORT        # 400 frame columns per core
PAIRS = [(t0, t1) for t0 in range(SEQ - 2) for t1 in range(t0 + 1, SEQ - 1)]
CMAX = 504                      # max score columns per group (9*56 <= 512)
BF16 = ml_dtypes.bfloat16

_CACHE = {}


def _pos_encoding():
    pos = np.arange(SEQ, dtype=np.float32)[:, None]
    div = np.exp(np.arange(0, IN_DIM, 2, dtype=np.float32) * -(math.log(10000.0) / IN_DIM))
    pe = np.zeros((SEQ, IN_DIM), dtype=np.float32)
    pe[:, 0::2] = np.sin(pos * div) * PE_SCALE
    pe[:, 1::2] = np.cos(pos * div) * PE_SCALE
    return pe


def _layout(counts):
    """Unpadded class-sorted row layout + per-(class,chunk) indicator pairs."""
    offs = [0]
    for c in range(WAY):
        offs.append(offs[-1] + int(counts[c]) * T)
    nb = offs[-1]                      # 1400
    nwch = (nb + 127) // 128           # 11
    nbp = nwch * 128                   # 1408
    pairs = []                         # (class, chunk, lo, hi) local partition range
    for c in range(WAY):
        r0, r1 = offs[c], offs[c + 1]
        for w in range(r0 // 128, (r1 + 127) // 128):
            lo = max(r0, w * 128) - w * 128
            hi = min(r1, (w + 1) * 128) - w * 128
            pairs.append((c, w, lo, hi))
    return offs, nb, nwch, nbp, pairs


def _group_sizes():
    gqs, rem = [], NQL
    while rem > 0:
        g = min(9, rem)
        gqs.append(g)
        rem -= g
    return gqs


def _build_kernel(counts, trivial_gb, trivial_bias):
    import concourse.mybir as mybir
    import concourse.tile as tile
    from concourse import bacc
    from concourse.masks import make_identity

    f32 = mybir.dt.float32
    bf16 = mybir.dt.bfloat16
    AF = mybir.ActivationFunctionType
    ALU = mybir.AluOpType

    offs, nb, nwch, nbp, pairs = _layout(counts)
    npair = len(pairs)
    inv_sqrt = 1.0 / math.sqrt(OUT_DIM)
    nkch = 16 if trivial_bias else 17
    gqs = _group_sizes()

    # per-class pair indices; chunk is "full" for protos iff every real s_v row
    # in it belongs to the class (pad rows at nb..nbp are zero in s_v)
    cls_pairs = {c: [] for c in range(WAY)}
    for p, (c, w, lo, hi) in enumerate(pairs):
        cls_pairs[c].append(p)
    full = {}
    for p, (c, w, lo, hi) in enumerate(pairs):
        real_hi = min(128, nb - w * 128)
        full[p] = (lo == 0 and hi >= real_hi)
    boundary = [p for p in range(npair) if not full[p]]

    nc = bacc.Bacc("TRN2", target_bir_lowering=False, debug=False,
                   enable_asserts=False, num_devices=N_CORES)

    x_d = nc.dram_tensor("x", [128, nkch, NX], bf16, kind="ExternalInput").ap()
    w_d = nc.dram_tensor("w", [128, NMB, nkch, 128], bf16, kind="ExternalInput").ap()
    g_d = nc.dram_tensor("lng", [128, NDCH], bf16, kind="ExternalInput").ap()
    b_d = nc.dram_tensor("lnb", [128, NDCH], bf16, kind="ExternalInput").ap()
    ind_d = nc.dram_tensor("ind", [128, npair], bf16, kind="ExternalInput").ap()
    out_d = nc.dram_tensor("out", [NQL, WAY], f32, kind="ExternalOutput").ap()

    with tile.TileContext(nc) as tc:
        with tc.tile_pool(name="big", bufs=1) as big, \
             tc.tile_pool(name="small", bufs=1) as small:
            s_kT = big.tile([128, NDCH, nbp], bf16)         # LN'd support K, T-layout
            s_v = big.tile([128, nwch, OUT_DIM], bf16)      # support V, row-natural
            svm = {p: big.tile([128, OUT_DIM], bf16, name=f"svm{p}")
                   for p in boundary}                       # masked boundary chunks
            q_kT = big.tile([128, NDCH, NQL, T], bf16)      # query K (pre-LN)
            q_vT = big.tile([128, NDCH, NQL, T], bf16)      # query V
            ones_sb = small.tile([128, 1], bf16)
            nc.vector.memset(ones_sb, 1.0)
            eps_sb = small.tile([1, 1], f32)
            nc.vector.memset(eps_sb, LN_EPS)
            g_sb = small.tile([128, NDCH], bf16)
            b_sb = small.tile([128, NDCH], bf16)
            ind_sb = small.tile([128, npair], bf16)
            nc.sync.dma_start(g_sb, g_d)
            nc.sync.dma_start(b_sb, b_d)
            nc.sync.dma_start(ind_sb, ind_d)
            logits5 = small.tile([WAY, NQL], f32)
            ident = small.tile([128, 128], bf16)
            make_identity(nc, ident)

            def packed_sum(ps_tile, slot, terms, first, last):
                """Accumulate sum-over-partitions of each (lhsT,rhs) term into
                ps_tile[32*slot] using a col-group tile_position."""
                out = ps_tile[32 * slot:32 * slot + 1]
                for i, (lhs, rhs) in enumerate(terms):
                    nc.tensor.matmul(out, lhs, rhs, start=(first and i == 0),
                                     stop=(last and i == len(terms) - 1),
                                     tile_position=(0, 32 * slot),
                                     skip_group_check=True)

            def col_ln(raw, out, cols, chunk, pool, psum_pool):
                """Column-wise LayerNorm of raw [128, NDCH, cols] (T-layout)."""
                for c0 in range(0, cols, chunk):
                    cw = min(chunk, cols - c0)
                    r = raw[:, :, c0:c0 + cw]
                    o = out[:, :, c0:c0 + cw]
                    sq = pool.tile([128, NDCH, chunk], bf16, tag="lnsq",
                                   name="lnsq", bufs=1)[:, :, :cw]
                    nc.scalar.activation(sq, r, AF.Square)
                    ps = psum_pool.tile([128, chunk], f32, tag="lnps",
                                        name="lnps")[:, :cw]
                    packed_sum(ps, 0, [(ones_sb, r[:, k]) for k in range(NDCH)],
                               True, True)
                    packed_sum(ps, 1, [(ones_sb, sq[:, k]) for k in range(NDCH)],
                               True, True)
                    m_r = pool.tile([1, chunk], f32, tag="lnm", name="lnm")[:, :cw]
                    v_r = pool.tile([1, chunk], f32, tag="lnv", name="lnv")[:, :cw]
                    mm = pool.tile([1, chunk], f32, tag="lnmm", name="lnmm")[:, :cw]
                    nc.scalar.activation(m_r, ps[0:1], AF.Copy, scale=1.0 / OUT_DIM)
                    nc.scalar.activation(v_r, ps[32:33], AF.Copy, scale=1.0 / OUT_DIM)
                    nc.vector.tensor_mul(mm, m_r, m_r)
                    nc.vector.tensor_sub(v_r, v_r, mm)
                    nc.scalar.activation(v_r, v_r, AF.Sqrt, bias=eps_sb)
                    nc.vector.reciprocal(v_r, v_r)
                    # bf16 broadcast operands keep the big apply passes in the
                    # DVE 16-bit fast path
                    m_h = pool.tile([1, chunk], bf16, tag="lnmh", name="lnmh")[:, :cw]
                    v_h = pool.tile([1, chunk], bf16, tag="lnvh", name="lnvh")[:, :cw]
                    nc.vector.tensor_copy(m_h, m_r)
                    nc.vector.tensor_copy(v_h, v_r)
                    m_b = pool.tile([128, chunk], bf16, tag="lnmb", name="lnmb",
                                    bufs=1)[:, :cw]
                    a_b = pool.tile([128, chunk], bf16, tag="lnab", name="lnab",
                                    bufs=1)[:, :cw]
                    nc.gpsimd.partition_broadcast(m_b, m_h)
                    nc.gpsimd.partition_broadcast(a_b, v_h)
                    mb3 = m_b[:, None, :].to_broadcast([128, NDCH, cw])
                    ab3 = a_b[:, None, :].to_broadcast([128, NDCH, cw])
                    nc.vector.tensor_sub(o, r, mb3)
                    nc.vector.tensor_mul(o, o, ab3)
                    if not trivial_gb:
                        for k in range(NDCH):
                            nc.vector.tensor_scalar(o[:, k], o[:, k],
                                                    g_sb[:, k:k + 1], b_sb[:, k:k + 1],
                                                    ALU.mult, ALU.add)

            # ---------- Phase 1: frame projections ----------
            f_cm = tc.tile_pool(name="fpool", bufs=1)
            f_pool = f_cm.__enter__()
            f_b = [f_pool.tile([128, NDCH, NX], bf16, name=f"f_b{j}")
                   for j in range(6)]
            with tc.tile_pool(name="xt_pool", bufs=1) as xt_pool, \
                 tc.tile_pool(name="xw", bufs=3) as xw, \
                 tc.tile_pool(name="pp_proj", bufs=4, space="PSUM") as pp_proj:
                xt = xt_pool.tile([128, nkch, NX], bf16)
                nc.sync.dma_start(xt, x_d)
                for m in range(NMB):
                    wm = xw.tile([128, nkch, 128], bf16, tag="wslab")
                    nc.sync.dma_start(wm, w_d[:, m])
                    ps = pp_proj.tile([128, NX], f32, tag="projps")
                    for k in range(nkch):
                        nc.tensor.matmul(ps, wm[:, k], xt[:, k],
                                         start=(k == 0), stop=(k == nkch - 1))
                    nc.scalar.activation(f_b[m // NDCH][:, m % NDCH], ps, AF.Copy)

            f_i = [fb.rearrange("p d (i s) -> p d i s", s=SEQ) for fb in f_b]

            def gather_one(dst4, kv, items0, n_items, pool, tag):
                """dst4 [128, NDCH, n_items, T] = tuple-gathered frame
                projections for one path (kv=0: K blocks 0-2, kv=1: V 3-5)."""
                isl = slice(items0, items0 + n_items)
                b0, b1, b2 = (f_i[3 * kv + j] for j in range(TSS))
                p2 = pool.tile([128, NDCH, N_SUPPORT, len(PAIRS)], bf16,
                               tag="p2", name="p2", bufs=1)[:, :, :n_items]
                pi = 0
                for t0 in range(SEQ - 2):
                    run = SEQ - 2 - t0
                    a = b0[:, :, isl, t0:t0 + 1]
                    b = b1[:, :, isl, t0 + 1:t0 + 1 + run]
                    nc.vector.tensor_add(p2[:, :, :, pi:pi + run],
                                         a.to_broadcast(b.shape), b)
                    pi += run
                ai = 0
                for pi2, (t0, t1) in enumerate(PAIRS):
                    run = SEQ - 1 - t1
                    a = p2[:, :, :, pi2:pi2 + 1]
                    b = b2[:, :, isl, t1 + 1:t1 + 1 + run]
                    nc.vector.tensor_add(dst4[:, :, :, ai:ai + run],
                                         a.to_broadcast(b.shape), b)
                    ai += run

            # ---------- Phase 2: support + query tensors ----------
            with tc.tile_pool(name="prep", bufs=1) as prep, \
                 tc.tile_pool(name="pp_prep", bufs=2, space="PSUM") as pp_prep, \
                 tc.tile_pool(name="pp_t", bufs=4, space="PSUM") as pp_t:
                # support K: gather straight into s_kT, LN in place
                nc.gpsimd.memset(s_kT[:, :, nb:nbp], 0.0)
                dst_k = s_kT[:, :, :nb].rearrange("p m (n a) -> p m n a", a=T)
                gather_one(dst_k, 0, 0, N_SUPPORT, prep, "sk")
                col_ln(s_kT, s_kT, nbp, CMAX, prep, pp_prep)
                # support V: gather in T-layout + transpose, staged in
                # 896-column halves (lcm(56,128)) to bound SBUF scratch
                H0 = 896 // T        # 16 items per full half
                for half, (i0, n_i) in enumerate([(0, H0), (H0, N_SUPPORT - H0)]):
                    ncols = n_i * T
                    ncp = ((ncols + 127) // 128) * 128
                    s_vT = prep.tile([128, NDCH, 896], bf16, tag="svt",
                                     name="svt", bufs=1)[:, :, :ncp]
                    if ncp > ncols:
                        nc.gpsimd.memset(s_vT[:, :, ncols:ncp], 0.0)
                    dst_v = s_vT[:, :, :ncols].rearrange(
                        "p m (n a) -> p m n a", a=T)
                    gather_one(dst_v, 1, i0, n_i, prep, "sv")
                    for wl in range(ncp // 128):
                        w = (i0 * T) // 128 + wl
                        for dd in range(NDCH):
                            ps = pp_t.tile([128, 128], bf16, tag="tps")
                            nc.tensor.transpose(
                                ps, s_vT[:, dd, wl * 128:(wl + 1) * 128], ident)
                            if dd % 2 == 0:
                                nc.vector.tensor_copy(
                                    s_v[:, w, dd * 128:(dd + 1) * 128], ps)
                            else:
                                nc.scalar.activation(
                                    s_v[:, w, dd * 128:(dd + 1) * 128], ps, AF.Copy)
                # masked copies of class-boundary chunks for the prototype path
                for p in boundary:
                    c, w, lo, hi = pairs[p]
                    nc.vector.tensor_mul(
                        svm[p], s_v[:, w],
                        ind_sb[:, p:p + 1].to_broadcast([128, OUT_DIM]))
                # query gathers + in-place LN of K (chunks aligned to the
                # phase-3 groups so each group depends only on its columns)
                gather_one(q_kT, 0, N_SUPPORT, NQL, prep, "qk")
                gather_one(q_vT, 1, N_SUPPORT, NQL, prep, "qv")
                qk3 = q_kT.rearrange("p m q a -> p m (q a)")
                qv3 = q_vT.rearrange("p m q a -> p m (q a)")
                col_ln(qk3, qk3, NQL * T, CMAX, prep, pp_prep)
            f_cm.__exit__(None, None, None)

            # ---------- Phase 3: per-group query pipeline ----------
            TS, TB, TC, TA = 0, 1, 2, 3  # term slots in the packed tile

            with tc.tile_pool(name="grp", bufs=2) as grp, \
                 tc.tile_pool(name="rows", bufs=2) as rows_pool, \
                 tc.tile_pool(name="pp_sc", bufs=2, space="PSUM") as pp_sc, \
                 tc.tile_pool(name="pp_pr", bufs=2, space="PSUM") as pp_pr, \
                 tc.tile_pool(name="pp_row", bufs=2, space="PSUM") as pp_row:
                q0 = 0
                for g_q in gqs:
                    C = g_q * T
                    c0 = q0 * T

                    # scoresT + exp
                    exp_t = grp.tile([128, nwch, CMAX], bf16, tag="exp", name="exp")[:, :, :C]
                    for w in range(nwch):
                        ps = pp_sc.tile([128, CMAX], f32, tag="scps", name="scps")[:, :C]
                        for k in range(NDCH):
                            nc.tensor.matmul(ps, s_kT[:, k, w * 128:(w + 1) * 128],
                                             qk3[:, k, c0:c0 + C], start=(k == 0),
                                             stop=(k == NDCH - 1))
                        nc.scalar.activation(exp_t[:, w], ps, AF.Exp, scale=inv_sqrt)

                    packed = rows_pool.tile([WAY, 4, CMAX], f32, tag="packed", name="packed")[:, :, :C]
                    qvg = qv3[:, :, c0:c0 + C]

                    # A = ||q_v||^2 per column
                    qsq = grp.tile([128, NDCH, CMAX], bf16, tag="qsq", name="qsq", bufs=1)[:, :, :C]
                    nc.scalar.activation(qsq, qvg, AF.Square)
                    ps_a = pp_row.tile([128, CMAX], f32, tag="sumps",
                                       name="ps_a")[:, :C]
                    packed_sum(ps_a, 0, [(ones_sb, qsq[:, k]) for k in range(NDCH)],
                               True, True)
                    a_sb = rows_pool.tile([1, CMAX], f32, tag="a_sb", name="a_sb", bufs=1)[:, :C]
                    nc.vector.tensor_copy(a_sb, ps_a[0:1])
                    a5 = rows_pool.tile([WAY, CMAX], f32, tag="a5", name="a5", bufs=1)[:, :C]
                    nc.gpsimd.partition_broadcast(a5, a_sb)

                    def s_terms(c):
                        return [(ind_sb[:, p:p + 1], exp_t[:, pairs[p][1]])
                                for p in cls_pairs[c]]

                    def stage_rows(ps, rows):
                        """PSUM sum-tile -> SBUF (one lane-parallel copy; engines
                        cannot address partition starts other than 0/32/64/96),
                        then DMA rows {32j} into packed[class, term]."""
                        st = rows_pool.tile([128, CMAX], f32, tag="stage",
                                            name="stage")[:, :C]
                        nc.scalar.activation(st, ps, AF.Copy)
                        st4 = st.rearrange("(j z) n -> j z n", z=32)
                        for j, (cc, term) in enumerate(rows):
                            nc.sync.dma_start(packed[cc:cc + 1, term],
                                              st4[j:j + 1, 0, :])

                    # S_0..S_3 packed in one PSUM tile (concurrent col-groups)
                    ps_s03 = pp_row.tile([128, CMAX], f32, tag="sumps",
                                         name="ps_s03")[:, :C]
                    nterms = max(len(s_terms(c)) for c in range(4))
                    for i in range(nterms):
                        for c in range(4):
                            tl = s_terms(c)
                            if i < len(tl):
                                packed_sum(ps_s03, c, [tl[i]], i == 0,
                                           i == len(tl) - 1)
                    stage_rows(ps_s03, [(0, TS), (1, TS), (2, TS), (3, TS)])

                    # remaining sum streams: S_4, then B_c/C_c per class,
                    # packed 4 per PSUM tile
                    ps_bc = pp_row.tile([128, CMAX], f32, tag="sumps",
                                        name="ps_bc0")[:, :C]
                    packed_sum(ps_bc, 0, s_terms(4), True, True)
                    pending = [(4, TS)]

                    def bc_flush(force=False):
                        nonlocal ps_bc, pending
                        if pending and (force or len(pending) >= 3):
                            stage_rows(ps_bc, pending)
                            pending = []
                            if not force:
                                ps_bc = pp_row.tile([128, CMAX], f32, tag="sumps",
                                                    name="ps_bc")[:, :C]

                    for c in range(WAY):
                        # prototypes: P[d, col] accumulated over class chunks
                        pt = grp.tile([128, NDCH, CMAX], bf16, tag="pt",
                                      name="pt", bufs=1)[:, :, :C]
                        p2t = grp.tile([128, NDCH, CMAX], bf16, tag="p2t",
                                       name="p2t", bufs=1)[:, :, :C]
                        chunks = [(s_v[:, pairs[p][1]] if full[p] else svm[p],
                                   pairs[p][1]) for p in cls_pairs[c]]
                        for dd in range(NDCH):
                            ps_p = pp_pr.tile([128, CMAX], f32, tag="prps", name="prps")[:, :C]
                            for wi, (lhs, w) in enumerate(chunks):
                                nc.tensor.matmul(ps_p,
                                                 lhs[:, dd * 128:(dd + 1) * 128],
                                                 exp_t[:, w], start=(wi == 0),
                                                 stop=(wi == len(chunks) - 1))
                            if dd % 2 == 0:
                                nc.scalar.activation(pt[:, dd], ps_p, AF.Copy)
                            else:
                                nc.vector.tensor_copy(pt[:, dd], ps_p)
                        nc.scalar.activation(p2t, pt, AF.Square)
                        nc.vector.tensor_mul(pt, pt, qvg)  # now <q_v, P> terms
                        # interleave B_c / C_c chunk streams for PE concurrency
                        sb, sc = len(pending), len(pending) + 1
                        for k in range(NDCH):
                            packed_sum(ps_bc, sb, [(ones_sb, pt[:, k])],
                                       k == 0, k == NDCH - 1)
                            packed_sum(ps_bc, sc, [(ones_sb, p2t[:, k])],
                                       k == 0, k == NDCH - 1)
                        pending += [(c, TB), (c, TC)]
                        bc_flush()
                    bc_flush(force=True)

                    # dist = A - 2 B/S + C/S^2 ; logits = -sum_a dist / T
                    sinv = rows_pool.tile([WAY, CMAX], f32, tag="sinv", name="sinv", bufs=1)[:, :C]
                    nc.vector.reciprocal(sinv, packed[:, TS])
                    u = rows_pool.tile([WAY, CMAX], f32, tag="u", name="u", bufs=1)[:, :C]
                    nc.vector.tensor_mul(u, packed[:, TC, :], sinv)
                    nc.vector.scalar_tensor_tensor(u, packed[:, TB, :], -2.0, u,
                                                   ALU.mult, ALU.add)
                    nc.vector.tensor_mul(u, u, sinv)
                    nc.vector.tensor_add(u, u, a5)
                    u4 = u.rearrange("w (q a) -> w q a", a=T)
                    red = rows_pool.tile([WAY, 9], f32, tag="red", name="red", bufs=1)[:, :g_q]
                    nc.vector.reduce_sum(red, u4, mybir.AxisListType.X)
                    nc.scalar.activation(logits5[:, q0:q0 + g_q], red,
                                         AF.Copy, scale=-1.0 / T)
                    q0 += g_q

            nc.sync.dma_start(out_d.rearrange("q c -> c q"), logits5)

    nc.compile()
    return nc


def kernel(support_set, support_labels, queries, k_w, k_b, v_w, v_b, ln_g, ln_b):
    import concourse.bass_utils as bass_utils

    support_set = np.asarray(support_set, dtype=np.float32)
    queries = np.asarray(queries, dtype=np.float32)
    labels = np.asarray(support_labels, dtype=np.int32)
    k_w = np.asarray(k_w, dtype=np.float32)
    v_w = np.asarray(v_w, dtype=np.float32)
    k_b = np.asarray(k_b, dtype=np.float32)
    v_b = np.asarray(v_b, dtype=np.float32)
    ln_g = np.asarray(ln_g, dtype=np.float32)
    ln_b = np.asarray(ln_b, dtype=np.float32)

    pe = _pos_encoding()
    s = support_set + pe[None]
    q = queries + pe[None]
    order = np.argsort(labels, kind="stable")
    counts = np.bincount(labels, minlength=WAY)
    s_sorted = s[order]
    trivial_gb = bool(np.all(ln_g == 1.0) and np.all(ln_b == 0.0))
    trivial_bias = bool(np.all(k_b == 0.0) and np.all(v_b == 0.0))
    nkch = 16 if trivial_bias else 17
    kpad = nkch * 128

    key = (tuple(int(x) for x in counts), trivial_gb, trivial_bias)
    if key not in _CACHE:
        _CACHE[key] = _build_kernel(counts, trivial_gb, trivial_bias)
    nc = _CACHE[key]

    _, nb, nwch, nbp, pairs = _layout(counts)
    npair = len(pairs)
    ind = np.zeros((128, npair), np.float32)
    for p, (c, w, lo, hi) in enumerate(pairs):
        ind[lo:hi, p] = 1.0
    ind = ind.astype(BF16)

    W = np.zeros((kpad, 6 * OUT_DIM), np.float32)
    for j in range(TSS):
        W[:IN_DIM, j * OUT_DIM:(j + 1) * OUT_DIM] = k_w[j * IN_DIM:(j + 1) * IN_DIM]
        W[:IN_DIM, (TSS + j) * OUT_DIM:(TSS + j + 1) * OUT_DIM] = v_w[j * IN_DIM:(j + 1) * IN_DIM]
        if not trivial_bias:
            W[IN_DIM, j * OUT_DIM:(j + 1) * OUT_DIM] = k_b / TSS
            W[IN_DIM, (TSS + j) * OUT_DIM:(TSS + j + 1) * OUT_DIM] = v_b / TSS
    w_perm = np.ascontiguousarray(
        W.reshape(nkch, 128, NMB, 128).transpose(1, 2, 0, 3)).astype(BF16)
    g_in = np.ascontiguousarray(ln_g.reshape(NDCH, 128).T).astype(BF16)
    b_in = np.ascontiguousarray(ln_b.reshape(NDCH, 128).T).astype(BF16)

    in_maps = []
    for core in range(N_CORES):
        qs = q[core * NQL:(core + 1) * NQL]
        X = np.concatenate([s_sorted.reshape(-1, IN_DIM), qs.reshape(-1, IN_DIM)], 0)
        XT = np.zeros((kpad, NX), np.float32)
        XT[:IN_DIM] = X.T
        if not trivial_bias:
            XT[IN_DIM] = 1.0
        x_perm = np.ascontiguousarray(
            XT.reshape(nkch, 128, NX).transpose(1, 0, 2)).astype(BF16)
        in_maps.append({"x": x_perm, "w": w_perm, "lng": g_in, "lnb": b_in,
                        "ind": ind})

    global _LAST_IN_MAPS
    _LAST_IN_MAPS = in_maps
    res = bass_utils.run_bass_kernel_spmd(nc, in_maps, core_ids=list(range(N_CORES)))
    return np.concatenate([res.results[i]["out"] for i in range(N_CORES)], 0)


_LAST_IN_MAPS = None


# revision 15
# speedup vs baseline: 1.1729x; 1.1729x over previous
"""Trainium2 Bass kernel for the CNN-TRX few-shot attention head.

Sharding: data-parallel over the 200 queries (25 per NeuronCore); support set
and weights replicated per core. All matmuls in bf16 with fp32 PSUM.

  1. Frame projection in transposed layout: f_T[d, frame] for all 6 weight
     blocks (k_w/v_w x 3 tuple positions); biases folded via an augmented
     ones-row of X only when biases are nonzero (16 vs 17 K-chunks).
  2. Tuple gather (C(8,3)=56 lex-ordered frame triples) as 2-stage DVE adds
     using the pair-suffix structure of lex combinations: stage 1 builds the
     21 (t1,t2) pair sums, stage 2 adds b0[t0] to the contiguous pair suffix
     (runs 21/15/10/6/3/1) - 12 instructions per gather. Query frames are
     laid out frame-major so the 25 items are the contiguous inner dim.
  3. Support rows class-sorted UNPADDED (1400 -> 1408 cols, 11 chunks).
     Per-class column sums use per-(class,chunk) indicator columns (input
     `ind`) as M=1 matmul lhsT; prototypes use raw s_v chunks plus
     mask-multiplied copies of the class-boundary chunks. Query score
     columns are tuple-major (a*NQL+q); per-group slices are strided APs.
  4. LayerNorm of K projections column-wise: stats via two ones-matmul
     streams packed into concurrent PE column groups, Rsqrt on ACT, gpsimd
     partition-broadcast, two DVE passes. Gather/LN scratch pools open
     BEFORE the projection pools so their SBUF does not alias xt/xw and the
     support-side prep overlaps the projection matmul stream.
  5. scoresT = s_k^T q_k per 128-row chunk; exp via ACT (no max-subtract:
     LN'd scores are bounded, exp stays finite in fp32).
  6. Query groups of 9/9/7 queries (504/504/392 score columns) pipelined
     through double-buffered PSUM/SBUF pools; distance terms ||q_v||^2,
     <q_v,P>, ||P||^2, sum(exp) via column-sum matmuls packed 4-per-PSUM
     tile at partitions {0,32,64,96}; final combine on 5 class lanes;
     logits = -sum_a dist / 56.
"""

import math
from itertools import combinations

import ml_dtypes
import numpy as np

SEQ = 8
IN_DIM = 2048
OUT_DIM = 1152
TSS = 3
WAY = 5
N_SUPPORT = 25
N_QUERIES = 200
PE_SCALE = 0.1
LN_EPS = 1e-5
T = 56
N_CORES = 8
NQL = N_QUERIES // N_CORES      # queries per core
NDCH = OUT_DIM // 128           # 9
NMB = 6 * OUT_DIM // 128        # 54 projection column blocks
NX = SEQ * 2 * N_SUPPORT        # 400 frame columns per core
CMAX = 504                      # max score columns per group (9*56 <= 512)
NPR = 21                        # C(7,2) (t1,t2) pairs
P2OFF = [0, 6, 11, 15, 18, 20]  # pair-block start for t1 = 1..6 (lex order)
BF16 = ml_dtypes.bfloat16

_CACHE = {}


def _pos_encoding():
    pos = np.arange(SEQ, dtype=np.float32)[:, None]
    div = np.exp(np.arange(0, IN_DIM, 2, dtype=np.float32) * -(math.log(10000.0) / IN_DIM))
    pe = np.zeros((SEQ, IN_DIM), dtype=np.float32)
    pe[:, 0::2] = np.sin(pos * div) * PE_SCALE
    pe[:, 1::2] = np.cos(pos * div) * PE_SCALE
    return pe


def _layout(counts):
    """Unpadded class-sorted row layout + per-(class,chunk) indicator pairs."""
    offs = [0]
    for c in range(WAY):
        offs.append(offs[-1] + int(counts[c]) * T)
    nb = offs[-1]                      # 1400
    nwch = (nb + 127) // 128           # 11
    nbp = nwch * 128                   # 1408
    pairs = []                         # (class, chunk, lo, hi) local partition range
    for c in range(WAY):
        r0, r1 = offs[c], offs[c + 1]
        for w in range(r0 // 128, (r1 + 127) // 128):
            lo = max(r0, w * 128) - w * 128
            hi = min(r1, (w + 1) * 128) - w * 128
            pairs.append((c, w, lo, hi))
    return offs, nb, nwch, nbp, pairs


def _group_sizes():
    gqs, rem = [], NQL
    while rem > 0:
        g = min(9, rem)
        gqs.append(g)
        rem -= g
    return gqs


def _build_kernel(counts, trivial_gb, trivial_bias):
    import concourse.mybir as mybir
    import concourse.tile as tile
    from concourse import bacc
    from concourse.masks import make_identity

    f32 = mybir.dt.float32
    bf16 = mybir.dt.bfloat16
    AF = mybir.ActivationFunctionType
    ALU = mybir.AluOpType

    offs, nb, nwch, nbp, pairs = _layout(counts)
    npair = len(pairs)
    inv_sqrt = 1.0 / math.sqrt(OUT_DIM)
    nkch = 16 if trivial_bias else 17
    gqs = _group_sizes()

    # per-class pair indices; chunk is "full" for protos iff every real s_v row
    # in it belongs to the class (pad rows at nb..nbp are zero in s_v)
    cls_pairs = {c: [] for c in range(WAY)}
    for p, (c, w, lo, hi) in enumerate(pairs):
        cls_pairs[c].append(p)
    full = {}
    for p, (c, w, lo, hi) in enumerate(pairs):
        real_hi = min(128, nb - w * 128)
        full[p] = (lo == 0 and hi >= real_hi)
    boundary = [p for p in range(npair) if not full[p]]

    nc = bacc.Bacc("TRN2", target_bir_lowering=False, debug=False,
                   enable_asserts=False, num_devices=N_CORES)

    x_d = nc.dram_tensor("x", [128, nkch, NX], bf16, kind="ExternalInput").ap()
    w_d = nc.dram_tensor("w", [128, NMB, nkch, 128], bf16, kind="ExternalInput").ap()
    g_d = nc.dram_tensor("lng", [128, NDCH], bf16, kind="ExternalInput").ap()
    b_d = nc.dram_tensor("lnb", [128, NDCH], bf16, kind="ExternalInput").ap()
    ind_d = nc.dram_tensor("ind", [128, npair], bf16, kind="ExternalInput").ap()
    out_d = nc.dram_tensor("out", [NQL, WAY], f32, kind="ExternalOutput").ap()

    with tile.TileContext(nc) as tc:
        with tc.tile_pool(name="big", bufs=1) as big, \
             tc.tile_pool(name="small", bufs=1) as small:
            s_kT = big.tile([128, NDCH, nbp], bf16)         # LN'd support K, T-layout
            s_v = big.tile([128, nwch, OUT_DIM], bf16)      # support V, row-natural
            svm = {p: big.tile([128, OUT_DIM], bf16, name=f"svm{p}")
                   for p in boundary}                       # masked boundary chunks
            q_kT = big.tile([128, NDCH, T, NQL], bf16)      # query K, tuple-major
            q_vT = big.tile([128, NDCH, T, NQL], bf16)      # query V, tuple-major
            ones_sb = small.tile([128, 1], bf16)
            nc.vector.memset(ones_sb, 1.0)
            eps_sb = small.tile([1, 1], f32)
            nc.vector.memset(eps_sb, LN_EPS)
            g_sb = small.tile([128, NDCH], bf16)
            b_sb = small.tile([128, NDCH], bf16)
            ind_sb = small.tile([128, npair], bf16)
            nc.sync.dma_start(g_sb, g_d)
            nc.sync.dma_start(b_sb, b_d)
            nc.sync.dma_start(ind_sb, ind_d)
            logits5 = small.tile([WAY, NQL], f32)
            ident = small.tile([128, 128], bf16)
            make_identity(nc, ident)

            def packed_sum(ps_tile, slot, terms, first, last):
                """Accumulate sum-over-partitions of each (lhsT,rhs) term into
                ps_tile[32*slot] using a col-group tile_position."""
                out = ps_tile[32 * slot:32 * slot + 1]
                for i, (lhs, rhs) in enumerate(terms):
                    nc.tensor.matmul(out, lhs, rhs, start=(first and i == 0),
                                     stop=(last and i == len(terms) - 1),
                                     tile_position=(0, 32 * slot),
                                     skip_group_check=True)

            def col_ln(raw, out, cols, chunk, pool, psum_pool):
                """Column-wise LayerNorm of raw [128, NDCH, cols] (T-layout)."""
                for c0 in range(0, cols, chunk):
                    cw = min(chunk, cols - c0)
                    r = raw[:, :, c0:c0 + cw]
                    o = out[:, :, c0:c0 + cw]
                    sq = pool.tile([128, NDCH, chunk], bf16, tag="lnsq",
                                   name="lnsq", bufs=1)[:, :, :cw]
                    nc.scalar.activation(sq, r, AF.Square)
                    ps = psum_pool.tile([128, chunk], f32, tag="lnps",
                                        name="lnps")[:, :cw]
                    packed_sum(ps, 0, [(ones_sb, r[:, k]) for k in range(NDCH)],
                               True, True)
                    packed_sum(ps, 1, [(ones_sb, sq[:, k]) for k in range(NDCH)],
                               True, True)
                    m_r = pool.tile([1, chunk], f32, tag="lnm", name="lnm")[:, :cw]
                    v_r = pool.tile([1, chunk], f32, tag="lnv", name="lnv")[:, :cw]
                    v_s = pool.tile([1, chunk], f32, tag="lnvs", name="lnvs")[:, :cw]
                    nc.scalar.activation(m_r, ps[0:1], AF.Copy, scale=1.0 / OUT_DIM)
                    nc.scalar.activation(v_r, ps[32:33], AF.Copy, scale=1.0 / OUT_DIM)
                    nc.vector.tensor_mul(v_s, m_r, m_r)
                    nc.vector.tensor_sub(v_r, v_r, v_s)
                    nc.scalar.activation(v_s, v_r, AF.Sqrt, bias=eps_sb)
                    nc.vector.reciprocal_approx_fast(v_r, v_s)
                    # bf16 broadcast operands keep the big apply passes in the
                    # DVE 16-bit fast path
                    m_h = pool.tile([1, chunk], bf16, tag="lnmh", name="lnmh")[:, :cw]
                    v_h = pool.tile([1, chunk], bf16, tag="lnvh", name="lnvh")[:, :cw]
                    nc.vector.tensor_copy(m_h, m_r)
                    nc.vector.tensor_copy(v_h, v_r)
                    m_b = pool.tile([128, chunk], bf16, tag="lnmb", name="lnmb",
                                    bufs=1)[:, :cw]
                    a_b = pool.tile([128, chunk], bf16, tag="lnab", name="lnab",
                                    bufs=1)[:, :cw]
                    nc.gpsimd.partition_broadcast(m_b, m_h)
                    nc.gpsimd.partition_broadcast(a_b, v_h)
                    mb3 = m_b[:, None, :].to_broadcast([128, NDCH, cw])
                    ab3 = a_b[:, None, :].to_broadcast([128, NDCH, cw])
                    nc.vector.tensor_sub(o, r, mb3)
                    nc.vector.tensor_mul(o, o, ab3)
                    if not trivial_gb:
                        for k in range(NDCH):
                            nc.vector.tensor_scalar(o[:, k], o[:, k],
                                                    g_sb[:, k:k + 1], b_sb[:, k:k + 1],
                                                    ALU.mult, ALU.add)

            # early pool: gather scratch, allocated BEFORE xt/xw so its
            # addresses never alias the projection inputs (no false WAR deps)
            # - the K-side gathers then overlap the projection matmul stream.
            # One shared 525-col tile serves both gather flavors (DVE executes
            # them serially anyway); LN scratch lives in the late pool since
            # the LN applies are gated on post-projection PE stats regardless.
            early_cm = tc.tile_pool(name="early", bufs=1)
            early = early_cm.__enter__()
            p2x = early.tile([128, NDCH, N_SUPPORT * NPR], bf16, name="p2x")
            p2s_v = p2x.rearrange("p d (n r) -> p d n r", r=NPR)
            p2q_v = p2x.rearrange("p d (r n) -> p d r n", n=NQL)
            f_cm = tc.tile_pool(name="fpool", bufs=1)
            f_pool = f_cm.__enter__()
            f_b = [f_pool.tile([128, NDCH, NX], bf16, name=f"f_b{j}")
                   for j in range(6)]

            # ---------- Phase 1: frame projections ----------
            with tc.tile_pool(name="xt_pool", bufs=1) as xt_pool, \
                 tc.tile_pool(name="xw", bufs=3) as xw, \
                 tc.tile_pool(name="pp_proj", bufs=4, space="PSUM") as pp_proj:
                xt = xt_pool.tile([128, nkch, NX], bf16)
                nc.sync.dma_start(xt, x_d)
                for m in range(NMB):
                    wm = xw.tile([128, nkch, 128], bf16, tag="wslab")
                    nc.sync.dma_start(wm, w_d[:, m])
                    ps = pp_proj.tile([128, NX], f32, tag="projps")
                    for k in range(nkch):
                        nc.tensor.matmul(ps, wm[:, k], xt[:, k],
                                         start=(k == 0), stop=(k == nkch - 1))
                    nc.scalar.activation(f_b[m // NDCH][:, m % NDCH], ps, AF.Copy)

            # support frames item-major (cols 0:200), query frames frame-major
            # (cols 200:400, items contiguous inner)
            fs_i = [fb[:, :, :SEQ * N_SUPPORT].rearrange(
                "p d (i s) -> p d i s", s=SEQ) for fb in f_b]
            fq_i = [fb[:, :, SEQ * N_SUPPORT:].rearrange(
                "p d (s q) -> p d s q", q=NQL) for fb in f_b]

            def gather_support(dst4, kv, items0, n_items):
                """dst4 [128, NDCH, n_items, 56] item-major tuple gather."""
                isl = slice(items0, items0 + n_items)
                b0, b1, b2 = (fs_i[3 * kv + j] for j in range(TSS))
                q2 = p2s_v[:, :, :n_items]
                for i1, t1 in enumerate(range(1, SEQ - 1)):
                    run = SEQ - 1 - t1
                    pi = P2OFF[i1]
                    b = b2[:, :, isl, t1 + 1:SEQ]
                    nc.vector.tensor_add(q2[:, :, :, pi:pi + run],
                                         b1[:, :, isl, t1:t1 + 1].to_broadcast(b.shape),
                                         b)
                ai = 0
                for t0 in range(SEQ - 2):
                    pi = P2OFF[t0]
                    run_p = NPR - pi
                    b = q2[:, :, :, pi:NPR]
                    nc.vector.tensor_add(dst4[:, :, :, ai:ai + run_p],
                                         b0[:, :, isl, t0:t0 + 1].to_broadcast(b.shape),
                                         b)
                    ai += run_p

            def gather_query(dst4, kv):
                """dst4 [128, NDCH, 56, 25] tuple-major gather, items inner."""
                b0, b1, b2 = (fq_i[3 * kv + j] for j in range(TSS))
                q2 = p2q_v
                for i1, t1 in enumerate(range(1, SEQ - 1)):
                    run = SEQ - 1 - t1
                    pi = P2OFF[i1]
                    b = b2[:, :, t1 + 1:SEQ, :]
                    nc.vector.tensor_add(q2[:, :, pi:pi + run, :],
                                         b1[:, :, t1:t1 + 1, :].to_broadcast(b.shape),
                                         b)
                ai = 0
                for t0 in range(SEQ - 2):
                    pi = P2OFF[t0]
                    run_p = NPR - pi
                    b = q2[:, :, pi:NPR, :]
                    nc.vector.tensor_add(dst4[:, :, ai:ai + run_p, :],
                                         b0[:, :, t0:t0 + 1, :].to_broadcast(b.shape),
                                         b)
                    ai += run_p

            # ---------- Phase 2: support + query tensors ----------
            with tc.tile_pool(name="late", bufs=1) as late, \
                 tc.tile_pool(name="pp_prep", bufs=2, space="PSUM") as pp_prep, \
                 tc.tile_pool(name="pp_t", bufs=4, space="PSUM") as pp_t:
                # K side first (only needs f_b[0..2], ready mid-projection)
                nc.gpsimd.memset(s_kT[:, :, nb:nbp], 0.0)
                dst_k = s_kT[:, :, :nb].rearrange("p m (n a) -> p m n a", a=T)
                gather_support(dst_k, 0, 0, N_SUPPORT)
                gather_query(q_kT, 0)
                col_ln(s_kT, s_kT, nbp, CMAX, late, pp_prep)
                qk3 = q_kT.rearrange("p m a q -> p m (a q)")
                col_ln(qk3, qk3, T * NQL, CMAX, late, pp_prep)
                # V side (needs f_b[3..5], ready at projection end); staged in
                # 896-column halves (lcm(56,128)) to bound SBUF scratch
                gather_query(q_vT, 1)
                H0 = 896 // T        # 16 items per full half
                for half, (i0, n_i) in enumerate([(0, H0), (H0, N_SUPPORT - H0)]):
                    ncols = n_i * T
                    ncp = ((ncols + 127) // 128) * 128
                    s_vT = late.tile([128, NDCH, 896], bf16, tag="svt",
                                     name="svt", bufs=1)[:, :, :ncp]
                    if ncp > ncols:
                        nc.gpsimd.memset(s_vT[:, :, ncols:ncp], 0.0)
                    dst_v = s_vT[:, :, :ncols].rearrange(
                        "p m (n a) -> p m n a", a=T)
                    gather_support(dst_v, 1, i0, n_i)
                    for wl in range(ncp // 128):
                        w = (i0 * T) // 128 + wl
                        for dd in range(NDCH):
                            ps = pp_t.tile([128, 128], bf16, tag="tps")
                            nc.tensor.transpose(
                                ps, s_vT[:, dd, wl * 128:(wl + 1) * 128], ident)
                            if dd % 2 == 0:
                                nc.vector.tensor_copy(
                                    s_v[:, w, dd * 128:(dd + 1) * 128], ps)
                            else:
                                nc.scalar.activation(
                                    s_v[:, w, dd * 128:(dd + 1) * 128], ps, AF.Copy)
                # masked copies of class-boundary chunks for the prototype path
                for p in boundary:
                    c, w, lo, hi = pairs[p]
                    nc.vector.tensor_mul(
                        svm[p], s_v[:, w],
                        ind_sb[:, p:p + 1].to_broadcast([128, OUT_DIM]))
            f_cm.__exit__(None, None, None)
            early_cm.__exit__(None, None, None)

            # ---------- Phase 3: per-group query pipeline ----------
            TS, TB, TC, TA = 0, 1, 2, 3  # term slots in the packed tile

            with tc.tile_pool(name="grp", bufs=2) as grp, \
                 tc.tile_pool(name="rows", bufs=2) as rows_pool, \
                 tc.tile_pool(name="pp_sc", bufs=2, space="PSUM") as pp_sc, \
                 tc.tile_pool(name="pp_pr", bufs=2, space="PSUM") as pp_pr, \
                 tc.tile_pool(name="pp_row", bufs=2, space="PSUM") as pp_row:
                q0 = 0
                for g_q in gqs:
                    C = g_q * T

                    # scoresT + exp
                    exp_t = grp.tile([128, nwch, CMAX], bf16, tag="exp",
                                     name="exp")[:, :, :C]
                    for w in range(nwch):
                        ps = pp_sc.tile([128, CMAX], f32, tag="scps",
                                        name="scps")[:, :C]
                        for k in range(NDCH):
                            nc.tensor.matmul(ps, s_kT[:, k, w * 128:(w + 1) * 128],
                                             q_kT[:, k, :, q0:q0 + g_q],
                                             start=(k == 0), stop=(k == NDCH - 1))
                        nc.scalar.activation(exp_t[:, w], ps, AF.Exp, scale=inv_sqrt)

                    packed = rows_pool.tile([WAY, 4, CMAX], f32, tag="packed",
                                            name="packed")[:, :, :C]
                    qvg = q_vT[:, :, :, q0:q0 + g_q]

                    # A = ||q_v||^2 per column
                    qsq = grp.tile([128, NDCH, CMAX], bf16, tag="qsq",
                                   name="qsq", bufs=1)[:, :, :C]
                    qsq4 = qsq.rearrange("p m (a q) -> p m a q", q=g_q)
                    nc.scalar.activation(qsq4, qvg, AF.Square)
                    ps_a = pp_row.tile([128, CMAX], f32, tag="sumps",
                                       name="ps_a")[:, :C]
                    packed_sum(ps_a, 0, [(ones_sb, qsq[:, k]) for k in range(NDCH)],
                               True, True)
                    a_sb = rows_pool.tile([1, CMAX], f32, tag="a_sb",
                                          name="a_sb", bufs=1)[:, :C]
                    nc.vector.tensor_copy(a_sb, ps_a[0:1])
                    a5 = rows_pool.tile([WAY, CMAX], f32, tag="a5",
                                        name="a5", bufs=1)[:, :C]
                    nc.gpsimd.partition_broadcast(a5, a_sb)

                    def s_terms(c):
                        return [(ind_sb[:, p:p + 1], exp_t[:, pairs[p][1]])
                                for p in cls_pairs[c]]

                    def stage_rows(ps, rows):
                        """PSUM sum-tile -> SBUF (one lane-parallel copy; engines
                        cannot address partition starts other than 0/32/64/96),
                        then DMA rows {32j} into packed[class, term]."""
                        st = rows_pool.tile([128, CMAX], f32, tag="stage",
                                            name="stage")[:, :C]
                        nc.scalar.activation(st, ps, AF.Copy)
                        st4 = st.rearrange("(j z) n -> j z n", z=32)
                        for j, (cc, term) in enumerate(rows):
                            nc.sync.dma_start(packed[cc:cc + 1, term],
                                              st4[j:j + 1, 0, :])

                    # S_0..S_3 packed in one PSUM tile (concurrent col-groups)
                    ps_s03 = pp_row.tile([128, CMAX], f32, tag="sumps",
                                         name="ps_s03")[:, :C]
                    nterms = max(len(s_terms(c)) for c in range(4))
                    for i in range(nterms):
                        for c in range(4):
                            tl = s_terms(c)
                            if i < len(tl):
                                packed_sum(ps_s03, c, [tl[i]], i == 0,
                                           i == len(tl) - 1)
                    stage_rows(ps_s03, [(0, TS), (1, TS), (2, TS), (3, TS)])

                    # remaining sum streams: S_4, then B_c/C_c per class,
                    # packed 4 per PSUM tile
                    ps_bc = pp_row.tile([128, CMAX], f32, tag="sumps",
                                        name="ps_bc0")[:, :C]
                    packed_sum(ps_bc, 0, s_terms(4), True, True)
                    pending = [(4, TS)]

                    def bc_flush(force=False):
                        nonlocal ps_bc, pending
                        if pending and (force or len(pending) >= 3):
                            stage_rows(ps_bc, pending)
                            pending = []
                            if not force:
                                ps_bc = pp_row.tile([128, CMAX], f32, tag="sumps",
                                                    name="ps_bc")[:, :C]

                    for c in range(WAY):
                        # prototypes: P[d, col] accumulated over class chunks
                        pt = grp.tile([128, NDCH, CMAX], bf16, tag="pt",
                                      name="pt", bufs=1)[:, :, :C]
                        p2t = grp.tile([128, NDCH, CMAX], bf16, tag="p2t",
                                       name="p2t", bufs=1)[:, :, :C]
                        chunks = [(s_v[:, pairs[p][1]] if full[p] else svm[p],
                                   pairs[p][1]) for p in cls_pairs[c]]
                        for dd in range(NDCH):
                            ps_p = pp_pr.tile([128, CMAX], f32, tag="prps",
                                              name="prps")[:, :C]
                            for wi, (lhs, w) in enumerate(chunks):
                                nc.tensor.matmul(ps_p,
                                                 lhs[:, dd * 128:(dd + 1) * 128],
                                                 exp_t[:, w], start=(wi == 0),
                                                 stop=(wi == len(chunks) - 1))
                            if dd % 2 == 0:
                                nc.scalar.activation(pt[:, dd], ps_p, AF.Copy)
                            else:
                                nc.vector.tensor_copy(pt[:, dd], ps_p)
                        nc.scalar.activation(p2t, pt, AF.Square)
                        pt4 = pt.rearrange("p m (a q) -> p m a q", q=g_q)
                        nc.vector.tensor_mul(pt4, pt4, qvg)  # now <q_v, P> terms
                        # interleave B_c / C_c chunk streams for PE concurrency
                        sb, sc = len(pending), len(pending) + 1
                        for k in range(NDCH):
                            packed_sum(ps_bc, sb, [(ones_sb, pt[:, k])],
                                       k == 0, k == NDCH - 1)
                            packed_sum(ps_bc, sc, [(ones_sb, p2t[:, k])],
                                       k == 0, k == NDCH - 1)
                        pending += [(c, TB), (c, TC)]
                        bc_flush()
                    bc_flush(force=True)

                    # dist = A - 2 B/S + C/S^2 ; logits = -sum_a dist / T
                    sinv = rows_pool.tile([WAY, CMAX], f32, tag="sinv",
                                          name="sinv", bufs=1)[:, :C]
                    nc.vector.reciprocal_approx_fast(sinv, packed[:, TS])
                    u = rows_pool.tile([WAY, CMAX], f32, tag="u",
                                       name="u", bufs=1)[:, :C]
                    nc.vector.tensor_mul(u, packed[:, TC, :], sinv)
                    nc.vector.scalar_tensor_tensor(u, packed[:, TB, :], -2.0, u,
                                                   ALU.mult, ALU.add)
                    nc.vector.tensor_mul(u, u, sinv)
                    nc.vector.tensor_add(u, u, a5)
                    u4 = u.rearrange("w (a q) -> w q a", q=g_q)
                    red = rows_pool.tile([WAY, 9], f32, tag="red",
                                         name="red", bufs=1)[:, :g_q]
                    nc.vector.reduce_sum(red, u4, mybir.AxisListType.X)
                    nc.scalar.activation(logits5[:, q0:q0 + g_q], red,
                                         AF.Copy, scale=-1.0 / T)
                    q0 += g_q

            nc.sync.dma_start(out_d.rearrange("q c -> c q"), logits5)

    nc.compile()
    return nc


def kernel(support_set, support_labels, queries, k_w, k_b, v_w, v_b, ln_g, ln_b):
    import concourse.bass_utils as bass_utils

    support_set = np.asarray(support_set, dtype=np.float32)
    queries = np.asarray(queries, dtype=np.float32)
    labels = np.asarray(support_labels, dtype=np.int32)
    k_w = np.asarray(k_w, dtype=np.float32)
    v_w = np.asarray(v_w, dtype=np.float32)
    k_b = np.asarray(k_b, dtype=np.float32)
    v_b = np.asarray(v_b, dtype=np.float32)
    ln_g = np.asarray(ln_g, dtype=np.float32)
    ln_b = np.asarray(ln_b, dtype=np.float32)

    pe = _pos_encoding()
    s = support_set + pe[None]
    q = queries + pe[None]
    order = np.argsort(labels, kind="stable")
    counts = np.bincount(labels, minlength=WAY)
    s_sorted = s[order]
    trivial_gb = bool(np.all(ln_g == 1.0) and np.all(ln_b == 0.0))
    trivial_bias = bool(np.all(k_b == 0.0) and np.all(v_b == 0.0))
    nkch = 16 if trivial_bias else 17
    kpad = nkch * 128

    key = (tuple(int(x) for x in counts), trivial_gb, trivial_bias)
    if key not in _CACHE:
        _CACHE[key] = _build_kernel(counts, trivial_gb, trivial_bias)
    nc = _CACHE[key]

    _, nb, nwch, nbp, pairs = _layout(counts)
    npair = len(pairs)
    ind = np.zeros((128, npair), np.float32)
    for p, (c, w, lo, hi) in enumerate(pairs):
        ind[lo:hi, p] = 1.0
    ind = ind.astype(BF16)

    W = np.zeros((kpad, 6 * OUT_DIM), np.float32)
    for j in range(TSS):
        W[:IN_DIM, j * OUT_DIM:(j + 1) * OUT_DIM] = k_w[j * IN_DIM:(j + 1) * IN_DIM]
        W[:IN_DIM, (TSS + j) * OUT_DIM:(TSS + j + 1) * OUT_DIM] = v_w[j * IN_DIM:(j + 1) * IN_DIM]
        if not trivial_bias:
            W[IN_DIM, j * OUT_DIM:(j + 1) * OUT_DIM] = k_b / TSS
            W[IN_DIM, (TSS + j) * OUT_DIM:(TSS + j + 1) * OUT_DIM] = v_b / TSS
    w_perm = np.ascontiguousarray(
        W.reshape(nkch, 128, NMB, 128).transpose(1, 2, 0, 3)).astype(BF16)
    g_in = np.ascontiguousarray(ln_g.reshape(NDCH, 128).T).astype(BF16)
    b_in = np.ascontiguousarray(ln_b.reshape(NDCH, 128).T).astype(BF16)

    in_maps = []
    for core in range(N_CORES):
        qs = q[core * NQL:(core + 1) * NQL]
        # query frames frame-major (items contiguous inner)
        Xq = np.ascontiguousarray(qs.transpose(1, 0, 2)).reshape(-1, IN_DIM)
        X = np.concatenate([s_sorted.reshape(-1, IN_DIM), Xq], 0)
        XT = np.zeros((kpad, NX), np.float32)
        XT[:IN_DIM] = X.T
        if not trivial_bias:
            XT[IN_DIM] = 1.0
        x_perm = np.ascontiguousarray(
            XT.reshape(nkch, 128, NX).transpose(1, 0, 2)).astype(BF16)
        in_maps.append({"x": x_perm, "w": w_perm, "lng": g_in, "lnb": b_in,
                        "ind": ind})

    global _LAST_IN_MAPS
    _LAST_IN_MAPS = in_maps
    res = bass_utils.run_bass_kernel_spmd(nc, in_maps, core_ids=list(range(N_CORES)))
    return np.concatenate([res.results[i]["out"] for i in range(N_CORES)], 0)


_LAST_IN_MAPS = None
